# revision 23
# baseline (speedup 1.0000x reference)
"""Cosine-similarity loss kernel for Trainium2 (8 NeuronCores, SPMD).

loss = -sum_n dot(s_n, im_n) / (||s_n|| * ||im_n||)   for s, im in R^{65536 x 512}

Strategy (HW-measured; per-core HBM ~358 GB/s, stream ~330 GB/s real):
  - Host casts inputs to fp16 (measured end-to-end rel err ~2.7e-4, well
    under the 2e-2 gate) -> 16.78 MB/core streamed instead of 33.55 MB,
    which makes the kernel compute-bound (stream ~51us, compute ~83us).
  - Rows sharded 8 ways; 64 slices of 128 rows per core.  All input DMA
    on the sync HWDGE ring (zero gaps at 4-8KB/partition descriptors).
  - Per slice, three one-pass reductions over [128, 512].  Every DVE
    reduce runs 1x (accumulator path is 1 elem/cycle regardless of
    dtype/claimed perf modes), so STT+accum is the cheapest DVE form
    (687+81 ns); ACT Square+accum is 708+278 ns.  GPSIMD shares DVE's
    2nd SBUF read port (exclusive lock) and the PE needs a transposed
    layout at 1/128 utilization -- both measured net losses.  Balance:
      dot -> DVE STT+accum (all 64)
      ss  -> ACT Square+accum (all 64)
      ii  -> DVE 44 / ACT 20  (DVE 108 ops ~83us, ACT 84 ops ~83us)
  - Tail: rsqrt(ss*ii) = ACT Sqrt(DVE reciprocal(ss*ii)); per-partition
    partials reduced across partitions with a PE ones-matmul into PSUM;
    single [1,1] f32 DMA out (a [128,1] out-DMA costs ~6.4us in 4B HBM
    read-modify-writes).
  - Host sums the 8 scalars.
"""

import numpy as np
from contextlib import ExitStack

import concourse.bacc as bacc
import concourse.bass as bass
import concourse.mybir as mybir
import concourse.tile as tile
from concourse.bass_utils import run_bass_kernel_spmd

N, D = 65536, 512
N_CORES = 8
ROWS = N // N_CORES          # 8192 rows per core
P = 128                      # SBUF partitions
SLICES = ROWS // P           # 64
F32 = mybir.dt.float32
F16 = mybir.dt.float16


def _build(
    rows=ROWS,
    seg_schedule=(1, 1, 2, 4, 8, 8, 8, 8, 8, 8, 4, 2, 1, 1),
    bufs=8,
    ii_on_act=lambda c: c % 16 >= 11,  # 20/64 ii on ACT: DVE 108, ACT 84 ops
):
    slices = rows // P
    assert sum(seg_schedule) == slices

    nc = bacc.Bacc(
        "TRN2", target_bir_lowering=False, debug=False, num_devices=N_CORES
    )
    s_d = nc.dram_tensor("s", [rows, D], F16, kind="ExternalInput").ap()
    im_d = nc.dram_tensor("im", [rows, D], F16, kind="ExternalInput").ap()
    out_d = nc.dram_tensor("out", [1, 1], F32, kind="ExternalOutput").ap()

    mult = mybir.AluOpType.mult
    add = mybir.AluOpType.add

    with tile.TileContext(nc) as tc, ExitStack() as ctx:
        spool = ctx.enter_context(tc.tile_pool(name="spool", bufs=bufs))
        ipool = ctx.enter_context(tc.tile_pool(name="ipool", bufs=bufs))
        qpool = ctx.enter_context(tc.tile_pool(name="qpool", bufs=4))
        stats = ctx.enter_context(tc.tile_pool(name="stats", bufs=1))
        ppool = ctx.enter_context(tc.psum_pool(name="ppool", bufs=1))

        dot_all = stats.tile([P, slices], F32)
        ss_all = stats.tile([P, slices], F32)
        ii_all = stats.tile([P, slices], F32)
        dve_scr = stats.tile([P, D], F16)
        act_scr = stats.tile([P, D], F16)
        ones = stats.tile([P, 1], F32)
        nc.vector.memset(ones[:], 1.0)
        # Dummy rsqrt-family op as ACT's first op: the compiler emits its
        # ACT_TABLE_LOAD here in the (DMA-ramp) head instead of on the
        # critical tail path.  Square is a cheap filler present in every
        # table set, so the mid-run squares need no reload.
        warm = stats.tile([1, 1], F32)
        nc.scalar.activation(
            warm[:], ones[0:1, :],
            mybir.ActivationFunctionType.Abs_reciprocal_sqrt,
        )

        c = 0
        r0 = 0
        for t, seg in enumerate(seg_schedule):
            nrows = seg * P
            s_seg = s_d[r0 : r0 + nrows, :].rearrange("(p j) d -> p j d", p=P, j=seg)
            im_seg = im_d[r0 : r0 + nrows, :].rearrange("(p j) d -> p j d", p=P, j=seg)
            r0 += nrows
            st = spool.tile([P, seg, D], F16, name="st", tag="st")
            nc.sync.dma_start(st[:], s_seg)
            it = ipool.tile([P, seg, D], F16, name="it", tag="it")
            nc.sync.dma_start(it[:], im_seg)

            for j in range(seg):
                cc = c + j
                # dot: DVE one-pass reduce (any DVE reduce runs 1x; STT is
                # the cheapest single-instruction form at 687+81 ns)
                nc.vector.scalar_tensor_tensor(
                    out=dve_scr[:], in0=st[:, j, :], scalar=1.0, in1=it[:, j, :],
                    op0=mult, op1=mult,
                    accum_out=dot_all[:, cc : cc + 1],
                )
                # ss: ACT
                nc.scalar.activation(
                    out=act_scr[:], in_=st[:, j, :],
                    func=mybir.ActivationFunctionType.Square,
                    accum_out=ss_all[:, cc : cc + 1],
                )
                # ii: split DVE/ACT to balance (DVE .768us/op, ACT .987us/op)
                if ii_on_act(cc):
                    nc.scalar.activation(
                        out=act_scr[:], in_=it[:, j, :],
                        func=mybir.ActivationFunctionType.Square,
                        accum_out=ii_all[:, cc : cc + 1],
                    )
                else:
                    nc.vector.scalar_tensor_tensor(
                        out=dve_scr[:], in0=it[:, j, :], scalar=1.0, in1=it[:, j, :],
                        op0=mult, op1=mult,
                        accum_out=ii_all[:, cc : cc + 1],
                    )
            c += seg

        # tail: loss_p[p] = -sum_c dot_c / sqrt(ss_c * ii_c)
        # rsqrt in ONE ACT op (prod > 0 so |x| is a no-op) -- one fewer
        # serial link than reciprocal+sqrt.
        prod = stats.tile([P, slices], F32)
        nc.vector.tensor_tensor(out=prod[:], in0=ss_all[:], in1=ii_all[:], op=mult)
        rsq = stats.tile([P, slices], F32)
        nc.scalar.activation(
            rsq[:], prod[:], mybir.ActivationFunctionType.Abs_reciprocal_sqrt,
        )
        fin_scr = stats.tile([P, slices], F32)
        loss_p = stats.tile([P, 1], F32)
        nc.vector.scalar_tensor_tensor(
            out=fin_scr[:], in0=dot_all[:], scalar=-1.0, in1=rsq[:],
            op0=mult, op1=mult,
            accum_out=loss_p[:],
        )
        # cross-partition reduce on the (idle) PE: ones^T @ loss_p -> [1,1]
        acc = ppool.tile([1, 1], F32)
        nc.tensor.matmul(acc[:], ones[:], loss_p[:])
        scal = stats.tile([1, 1], F32)
        nc.scalar.copy(scal[:], acc[:])
        nc.sync.dma_start(out_d, scal[:])

    nc.compile()
    return nc


_compiled = None


def _get_nc():
    global _compiled
    if _compiled is None:
        _compiled = _build()
    return _compiled


def _run(s, im, nc=None, **kw):
    """Cast fp16, shard, run on 8 cores, return BassKernelResults."""
    s16 = np.ascontiguousarray(np.asarray(s, dtype=np.float32).astype(np.float16))
    im16 = np.ascontiguousarray(np.asarray(im, dtype=np.float32).astype(np.float16))
    assert s16.shape == (N, D) and im16.shape == (N, D)
    if nc is None:
        nc = _get_nc()
    in_maps = [
        {"s": s16[c * ROWS : (c + 1) * ROWS], "im": im16[c * ROWS : (c + 1) * ROWS]}
        for c in range(N_CORES)
    ]
    bkr = run_bass_kernel_spmd(nc, in_maps, core_ids=list(range(N_CORES)), **kw)
    return bkr


def kernel(s, im, temp=None, **_):
    bkr = _run(s, im)
    total = np.float64(0.0)
    for r in bkr.results:
        total += np.float64(r["out"].reshape(-1)[0])
    return np.float32(total)
